# revision 32
# baseline (speedup 1.0000x reference)
import numpy as np
import ml_dtypes
from contextlib import ExitStack

import concourse.mybir as mybir
import concourse.bass as bass
import concourse.tile as tile
from concourse.bass_utils import run_bass_kernel_spmd

# Problem: nn_Predictor (moe_routing). L=6 streams, B=16384, D=512, NC=3992, 4 experts.
#
# Host computes the hard gate (argmax of fp64 logits) and expert-sorts tokens;
# each core gets 128-aligned per-expert slot counts (identical across cores ->
# one SPMD program); tokens that don't fit run on the host (<3%). Each token
# only runs through its own expert. Inputs arrive pre-transposed feature-major.
#
# Device pipeline per 512-token tile:
#   W1 (fp8 DoubleRow) -> relu -> fused W2@dec_W1 (fp8 DoubleRow) -> sigmoid
#   -> dec2 (bf16) -> out (bf16, +dec_b2 added on host)
# The two decoder linears are fused host-side (no nonlinearity between them).
# fp8 operands carry power-of-2 scales folded into the weights; the scales are
# removed for free via the activation instructions' `scale` argument.
L, B, D, NCLS, NE = 6, 16384, 512, 3992, 4
NCORES = 8
NDEV = 2048                 # device tokens per core (4 tiles of 512)
TT = 512
NT = NDEV // TT
NK = [12, 12, 24, 24]       # 128-wide K chunks of W1 per expert
NP = [k // 2 for k in NK]   # 256-wide DoubleRow K pairs
PLO = [0, 6, 0, 0]          # first fusionT chunk-pair each expert reads
NCH = (NCLS + 511) // 512   # 8 output column chunks (last = 408)

XS, WS, HS, MS = 16.0, 64.0, 32.0, 128.0   # fp8 scales: x, W1, h, W2@dW1

F32 = mybir.dt.float32
BF16 = mybir.dt.bfloat16
F8 = mybir.dt.float8e4
bf16 = ml_dtypes.bfloat16
f8 = ml_dtypes.float8_e4m3fn
AF = mybir.ActivationFunctionType
DR = mybir.MatmulPerfMode.DoubleRow


def _build(bounds):
    """bounds: cumulative token boundaries of the 4 expert segments in each
    core's 2048-token stream (identical for every core)."""
    nc = bass.Bass("TRN2")

    fusionT = nc.dram_tensor("fusionT", [L * D, NDEV], F8, kind="ExternalInput")
    w1s = [
        nc.dram_tensor(f"w1s{e}", [128, NP[e] * 1024], F8, kind="ExternalInput")
        for e in range(NE)
    ]
    w2s = nc.dram_tensor("w2s", [128, NE * 2048], F8, kind="ExternalInput")
    dw2s = nc.dram_tensor("dw2s", [128, NCH * 2048], BF16, kind="ExternalInput")
    b1b = nc.dram_tensor("b1b", [128, NE * 4], F32, kind="ExternalInput")
    cb = nc.dram_tensor("cb", [128, NE * 4], F32, kind="ExternalInput")
    out = nc.dram_tensor("out", [NDEV, NCLS], BF16, kind="ExternalOutput")

    segs = []
    prev = 0
    for e, b in enumerate(bounds):
        if b > prev:
            segs.append((e, prev, b))
            prev = b
    tile_segs = []
    for t in range(NT):
        t0, t1 = t * TT, (t + 1) * TT
        tile_segs.append(
            [(e, max(lo, t0) - t0, min(hi, t1) - t0) for (e, lo, hi) in segs
             if lo < t1 and hi > t0]
        )

    with tile.TileContext(nc) as tc, ExitStack() as ctx:
        singles = ctx.enter_context(tc.tile_pool(name="singles", bufs=1))
        htP = ctx.enter_context(tc.tile_pool(name="htP", bufs=6))
        sigP = ctx.enter_context(tc.tile_pool(name="sigP", bufs=6))
        outP = ctx.enter_context(tc.tile_pool(name="outP", bufs=6))

        psA = ctx.enter_context(tc.tile_pool(name="psA", bufs=2, space="PSUM"))
        psB = ctx.enter_context(tc.tile_pool(name="psB", bufs=4, space="PSUM"))
        psC = ctx.enter_context(tc.tile_pool(name="psC", bufs=2, space="PSUM"))

        # every 256-feature chunk pair is loaded once, whole-row (all 2048
        # tokens), and stays resident; all tiles slice their token columns.
        # Loads are merged into block DMAs (each dma_start costs ~650ns of
        # engine issue time) and column-halved so tiles 0/1 start early.
        NPAIR = L * D // 256
        flat_all = singles.tile([128, NPAIR, 2, NDEV], F8)

        def load_block(p0, p1, q):
            # pairs [p0,p1) x token columns [q*TT, (q+1)*TT) only
            nc.sync.dma_start(
                out=flat_all[:, p0:p1, :, q * TT: (q + 1) * TT],
                in_=bass.AP(
                    tensor=fusionT, offset=p0 * 256 * NDEV + q * TT,
                    ap=[[NDEV, 128], [256 * NDEV, p1 - p0], [128 * NDEV, 2],
                        [1, TT]]),
            )

        # sync ring: W1 weights + input rows in first-use order, then all
        # output writes. scalar ring: biases, fused W2, first dw2 chunks;
        # the rest of dw2 streams in from inside dec2(0) (time-gated).
        w1sb = [singles.tile([128, NP[e], 2, 4, 128], F8, name=f"w1sb{e}")
                for e in range(NE)]
        b1sb = singles.tile([128, NE * 4], F32)
        nc.scalar.dma_start(out=b1sb, in_=b1b[:, :])
        cbsb = singles.tile([128, NE * 4], F32)
        nc.scalar.dma_start(out=cbsb, in_=cb[:, :])
        w2sb = singles.tile([128, NE, 2, 2, 4, 128], F8)
        nc.scalar.dma_start(out=w2sb, in_=w2s[:, :])
        dw2sb = singles.tile([128, NCH, 4, 512], BF16)
        for n in range(2):
            nc.scalar.dma_start(out=dw2sb[:, n, :, :],
                                in_=dw2s[:, n * 2048: (n + 1) * 2048])

        loaded_w, loaded_q = set(), set()

        def load_missing(pairs, q, max_block):
            run = []
            for p in sorted(pairs):
                if (p, q) in loaded_q:
                    continue
                loaded_q.add((p, q))
                if run and (p != run[-1] + 1 or len(run) >= max_block):
                    load_block(run[0], run[-1] + 1, q)
                    run = []
                run.append(p)
            if run:
                load_block(run[0], run[-1] + 1, q)

        e0_ = tile_segs[0][0][0]
        nc.sync.dma_start(out=w1sb[e0_][:, :1], in_=w1s[e0_][:, :1024])
        load_missing(range(PLO[e0_], PLO[e0_] + 2), 0, 2)
        nc.sync.dma_start(out=w1sb[e0_][:, 1:], in_=w1s[e0_][:, 1024:])
        load_missing(range(PLO[e0_], PLO[e0_] + NP[e0_]), 0, 2)
        loaded_w.add(e0_)
        for t in range(1, NT):
            for (e, _, _) in tile_segs[t]:
                if e not in loaded_w:
                    nc.sync.dma_start(out=w1sb[e], in_=w1s[e][:, :])
                    loaded_w.add(e)
                load_missing(range(PLO[e], PLO[e] + NP[e]), t, NPAIR)
        for e in range(NE):
            if e not in loaded_w:
                nc.sync.dma_start(out=w1sb[e], in_=w1s[e][:, :])

        sig_tiles = [None] * NT

        def emit_front(t):
            """W1 -> relu -> fused (W2 @ dec_W1) -> sigmoid for tile t."""
            selB = [psB.tile([128, TT], F32, name="selps") for _ in range(4)]
            for (e, lo, hi) in tile_segs[t]:
                w = hi - lo
                glo = t * TT + lo
                hpair = [htP.tile([128, 2, TT], F8, name="ht") for _ in range(2)]
                for m in range(4):
                    hps = psA.tile([128, TT], F32, name="hps")
                    for kp in range(NP[e]):
                        nc.tensor.matmul(
                            hps[:, :w],
                            w1sb[e][:, kp, :, m, :],
                            flat_all[:, PLO[e] + kp, :, glo: glo + w],
                            start=(kp == 0),
                            stop=(kp == NP[e] - 1),
                            perf_mode=DR,
                        )
                    # psum holds XS*WS*(x@W1); write HS*relu(x@W1 + b1) as fp8
                    nc.scalar.activation(
                        hpair[m // 2][:, m % 2, :w], hps[:, :w], AF.Relu,
                        bias=b1sb[:, e * 4 + m: e * 4 + m + 1],
                        scale=HS / (XS * WS),
                    )
                    if m % 2 == 1:
                        kp = m // 2
                        for md in range(4):
                            nc.tensor.matmul(
                                selB[md][:, lo:hi],
                                w2sb[:, e, kp, :, md, :],
                                hpair[kp][:, :, :w],
                                start=(kp == 0),
                                stop=(kp == 1),
                                perf_mode=DR,
                            )
            sigT = []
            for md in range(4):
                sg = sigP.tile([128, TT], BF16, name="sg")
                for (e, lo, hi) in tile_segs[t]:
                    nc.scalar.activation(
                        sg[:, lo:hi], selB[md][:, lo:hi], AF.Sigmoid,
                        bias=cbsb[:, e * 4 + md: e * 4 + md + 1],
                        scale=1.0 / (HS * MS),
                    )
                sigT.append(sg)
            sig_tiles[t] = sigT

        def emit_dec2(t):
            sigT = sig_tiles[t]
            for n in range(NCH):
                nw = min(512, NCLS - n * 512)
                if t == 0 and n + 2 < NCH:
                    # stream the rest of dw2 in from inside the first dec2
                    # pass: the scalar queue only reaches this once the
                    # preceding output writes fire, so these loads cannot
                    # steal HBM bandwidth from the startup-critical loads.
                    nc.scalar.dma_start(
                        out=dw2sb[:, n + 2, :, :],
                        in_=dw2s[:, (n + 2) * 2048: (n + 3) * 2048])
                ot = outP.tile([128, 4, 512], BF16, name="ot")
                eng = nc.sync if n % 2 == 0 else nc.scalar
                for s in range(TT // 128):
                    ps = psC.tile([128, 512], F32, name="d2ps")
                    for kh in range(4):
                        nc.tensor.matmul(
                            ps[:, :nw],
                            sigT[kh][:, s * 128: (s + 1) * 128],
                            dw2sb[:, n, kh, :nw],
                            start=(kh == 0),
                            stop=(kh == 3),
                        )
                    nc.vector.tensor_copy(out=ot[:, s, :nw], in_=ps[:, :nw])
                    if t == NT - 1:
                        # last tile: per-chunk writes so the final drain only
                        # trails the last cast by one small transfer
                        eng.dma_start(
                            out=out[t * TT + s * 128: t * TT + (s + 1) * 128,
                                    n * 512: n * 512 + nw],
                            in_=ot[:, s, :nw],
                        )
                if t < NT - 1:
                    # one merged write per (tile, column chunk)
                    eng.dma_start(
                        out=bass.AP(tensor=out,
                                    offset=(t * TT) * NCLS + n * 512,
                                    ap=[[NCLS, 128], [128 * NCLS, 4], [1, nw]]),
                        in_=ot[:, :, :nw],
                    )

        # dec2(t) directly after front(t): dec2 is a long dense PE stretch
        # whose inputs are already on-chip, giving the DMA rings a wide
        # window to stage the next tile's weights/activations.
        for t in range(NT):
            emit_front(t)
            emit_dec2(t)

    import bass_rust

    bass_rust.generate_event_semaphores(nc)
    return nc


_NC_CACHE = {}


def _get_nc(bounds):
    key = tuple(bounds)
    if key not in _NC_CACHE:
        _NC_CACHE[key] = _build(key)
    return _NC_CACHE[key]


def _swizzle_dr(w, s):
    """[K, M] f32 -> DoubleRow lhsT layout [128, (K/256)*2*(M/128)*128] fp8:
    element [p, kp, r, m, j] = s * w[(2*kp+r)*128+p, m*128+j]."""
    K, M = w.shape
    return np.ascontiguousarray(
        (w * s).reshape(K // 256, 2, 128, M // 128, 128)
        .transpose(2, 0, 1, 3, 4).reshape(128, (K // 128) * M).astype(f8)
    )


def _prep(inputs):
    f32 = np.float32
    x = np.asarray(inputs["fusion_hs"], f32)                 # [L, B, D]
    flat = np.ascontiguousarray(np.transpose(x, (1, 0, 2)).reshape(B, L * D))

    logits = flat.astype(np.float64) @ np.asarray(inputs["gate_W"], f32).astype(
        np.float64
    ) + np.asarray(inputs["gate_b"], f32).astype(np.float64)
    am = np.argmax(logits, axis=1)

    idx = [np.nonzero(am == e)[0] for e in range(NE)]
    # per-core per-expert slot counts: multiples of 128 summing to NDEV,
    # largest-remainder so tile boundaries mostly align with expert
    # boundaries. Tokens that don't fit their expert's slots run on the host.
    want = [len(idx[e]) / NCORES / 128 for e in range(NE)]
    ke = [int(w) for w in want]
    while sum(ke) < NDEV // 128:
        ke[max(range(NE), key=lambda e: want[e] - ke[e])] += 1
    ke = [k * 128 for k in ke]
    ke_dev = [min(ke[e], len(idx[e]) // NCORES) for e in range(NE)]
    bounds = (ke[0], ke[0] + ke[1], ke[0] + ke[1] + ke[2], NDEV)

    w1_3s = np.array(inputs["e3_W1"], f32, copy=True)
    w1_3s[: 3 * D] *= f32(np.asarray(inputs["e3_a"]).reshape(-1)[0])
    w1_3s[3 * D:] *= f32(np.asarray(inputs["e3_b"]).reshape(-1)[0])
    W1 = [np.asarray(inputs["e0_W1"], f32), np.asarray(inputs["e1_W1"], f32),
          np.asarray(inputs["e2_W1"], f32), w1_3s]
    W2 = [np.asarray(inputs[f"e{e}_W2"], f32) for e in range(NE)]
    b1 = [np.asarray(inputs[f"e{e}_b1"], f32) for e in range(NE)]
    b2 = [np.asarray(inputs[f"e{e}_b2"], f32) for e in range(NE)]
    dW1 = np.asarray(inputs["dec_W1"], f32)
    db1 = np.asarray(inputs["dec_b1"], f32)
    dW2 = np.asarray(inputs["dec_W2"], f32)
    db2 = np.asarray(inputs["dec_b2"], f32)

    # fuse the two back-to-back linear layers: sel@dW1 = h@(W2@dW1) + (b2@dW1)
    M2 = [W2[e] @ dW1 for e in range(NE)]
    cbv = [b2[e] @ dW1 + db1 for e in range(NE)]
    dW2p = np.zeros((512, NCH * 512), f32)
    dW2p[:, :NCLS] = dW2
    common = {
        "w1s0": _swizzle_dr(W1[0], WS), "w1s1": _swizzle_dr(W1[1], WS),
        "w1s2": _swizzle_dr(W1[2], WS), "w1s3": _swizzle_dr(W1[3], WS),
        "w2s": np.concatenate([_swizzle_dr(m, MS) for m in M2], axis=1),
        "dw2s": np.ascontiguousarray(
            dW2p.reshape(4, 128, NCH, 512).transpose(1, 2, 0, 3)
            .reshape(128, NCH * 2048).astype(bf16)
        ),
        # relu activation computes relu(psum/(XS*WS/HS)... ) with bias HS*b1
        "b1b": np.stack([HS * b1[e][m * 128: (m + 1) * 128]
                         for e in range(NE) for m in range(4)], axis=1),
        "cb": np.stack([cbv[e][m * 128: (m + 1) * 128]
                        for e in range(NE) for m in range(4)], axis=1),
    }

    perms, in_maps = [], []
    for c in range(NCORES):
        src = np.full(NDEV, -1, np.int64)
        base = 0
        for e in range(NE):
            k = ke_dev[e]
            src[base: base + k] = idx[e][c * k: (c + 1) * k]
            base += ke[e]
        perms.append(src)
        valid = src >= 0
        a = np.zeros((NDEV, L * D), f8)
        a[valid] = (flat[src[valid]] * XS).astype(f8)
        m = dict(common)
        m["fusionT"] = np.ascontiguousarray(a.T)
        in_maps.append(m)

    # overflow tokens: full forward on host in fp32
    lt = np.concatenate([idx[e][NCORES * ke_dev[e]:] for e in range(NE)])
    lt_out = np.zeros((len(lt), NCLS), f32)
    if len(lt):
        off = 0
        ins = [flat[:, : 3 * D], flat[:, 3 * D:], flat, flat]
        for e in range(NE):
            g = idx[e][NCORES * ke_dev[e]:]
            if len(g) == 0:
                continue
            h = np.maximum(ins[e][g] @ W1[e] + b1[e], 0)
            sel = h @ W2[e] + b2[e]
            sig = 1.0 / (1.0 + np.exp(-(sel @ dW1 + db1)))
            lt_out[off: off + len(g)] = sig @ dW2 + db2
            off += len(g)

    return bounds, in_maps, perms, lt, lt_out, db2


def kernel(**inputs):
    bounds, in_maps, perms, lt, lt_out, db2 = _prep(inputs)
    nc = _get_nc(bounds)
    res = run_bass_kernel_spmd(nc, in_maps, core_ids=list(range(NCORES)))
    out = np.empty((B, NCLS), np.float32)
    for c in range(NCORES):
        dev = np.asarray(res.results[c]["out"], np.float32)
        valid = perms[c] >= 0
        out[perms[c][valid]] = dev[valid] + db2
    if len(lt):
        out[lt] = lt_out
    return out


# revision 33
# speedup vs baseline: 1.0016x; 1.0016x over previous
import numpy as np
import ml_dtypes
from contextlib import ExitStack

import concourse.mybir as mybir
import concourse.bass as bass
import concourse.tile as tile
from concourse.bass_utils import run_bass_kernel_spmd

# Problem: nn_Predictor (moe_routing). L=6 streams, B=16384, D=512, NC=3992, 4 experts.
#
# Host computes the hard gate (argmax of fp64 logits) and expert-sorts tokens;
# each core gets 128-aligned per-expert slot counts (identical across cores ->
# one SPMD program); tokens that don't fit run on the host (<3%). Each token
# only runs through its own expert. Inputs arrive pre-transposed feature-major.
#
# Device pipeline per 512-token tile:
#   W1 (fp8 DoubleRow) -> relu -> fused W2@dec_W1 (fp8 DoubleRow) -> sigmoid
#   -> dec2 (bf16) -> out (bf16, +dec_b2 added on host)
# The two decoder linears are fused host-side (no nonlinearity between them).
# fp8 operands carry power-of-2 scales folded into the weights; the scales are
# removed for free via the activation instructions' `scale` argument.
L, B, D, NCLS, NE = 6, 16384, 512, 3992, 4
NCORES = 8
NDEV = 2048                 # device tokens per core (4 tiles of 512)
TT = 512
NT = NDEV // TT
NK = [12, 12, 24, 24]       # 128-wide K chunks of W1 per expert
NP = [k // 2 for k in NK]   # 256-wide DoubleRow K pairs
PLO = [0, 6, 0, 0]          # first fusionT chunk-pair each expert reads
NCH = (NCLS + 511) // 512   # 8 output column chunks (last = 408)

XS, WS, HS, MS = 16.0, 64.0, 32.0, 128.0   # fp8 scales: x, W1, h, W2@dW1

F32 = mybir.dt.float32
BF16 = mybir.dt.bfloat16
F8 = mybir.dt.float8e4
bf16 = ml_dtypes.bfloat16
f8 = ml_dtypes.float8_e4m3fn
AF = mybir.ActivationFunctionType
DR = mybir.MatmulPerfMode.DoubleRow


def _build(bounds):
    """bounds: cumulative token boundaries of the 4 expert segments in each
    core's 2048-token stream (identical for every core)."""
    nc = bass.Bass("TRN2")

    fusionT = nc.dram_tensor("fusionT", [L * D, NDEV], F8, kind="ExternalInput")
    w1s = [
        nc.dram_tensor(f"w1s{e}", [128, NP[e] * 1024], F8, kind="ExternalInput")
        for e in range(NE)
    ]
    w2s = nc.dram_tensor("w2s", [128, NE * 2048], F8, kind="ExternalInput")
    dw2s = nc.dram_tensor("dw2s", [128, NCH * 2048], BF16, kind="ExternalInput")
    b1b = nc.dram_tensor("b1b", [128, NE * 4], F32, kind="ExternalInput")
    cb = nc.dram_tensor("cb", [128, NE * 4], F32, kind="ExternalInput")
    out = nc.dram_tensor("out", [NDEV, NCLS], BF16, kind="ExternalOutput")

    segs = []
    prev = 0
    for e, b in enumerate(bounds):
        if b > prev:
            segs.append((e, prev, b))
            prev = b
    tile_segs = []
    for t in range(NT):
        t0, t1 = t * TT, (t + 1) * TT
        tile_segs.append(
            [(e, max(lo, t0) - t0, min(hi, t1) - t0) for (e, lo, hi) in segs
             if lo < t1 and hi > t0]
        )

    with tile.TileContext(nc) as tc, ExitStack() as ctx:
        singles = ctx.enter_context(tc.tile_pool(name="singles", bufs=1))
        htP = ctx.enter_context(tc.tile_pool(name="htP", bufs=6))
        sigP = ctx.enter_context(tc.tile_pool(name="sigP", bufs=6))
        outP = ctx.enter_context(tc.tile_pool(name="outP", bufs=6))

        psA = ctx.enter_context(tc.tile_pool(name="psA", bufs=2, space="PSUM"))
        psB = ctx.enter_context(tc.tile_pool(name="psB", bufs=4, space="PSUM"))
        psC = ctx.enter_context(tc.tile_pool(name="psC", bufs=2, space="PSUM"))

        # every 256-feature chunk pair is loaded once, whole-row (all 2048
        # tokens), and stays resident; all tiles slice their token columns.
        # Loads are merged into block DMAs (each dma_start costs ~650ns of
        # engine issue time) and column-halved so tiles 0/1 start early.
        NPAIR = L * D // 256
        flat_all = singles.tile([128, NPAIR, 2, NDEV], F8)

        def load_block(p0, p1, q):
            # pairs [p0,p1) x token columns [q*TT, (q+1)*TT) only
            nc.sync.dma_start(
                out=flat_all[:, p0:p1, :, q * TT: (q + 1) * TT],
                in_=bass.AP(
                    tensor=fusionT, offset=p0 * 256 * NDEV + q * TT,
                    ap=[[NDEV, 128], [256 * NDEV, p1 - p0], [128 * NDEV, 2],
                        [1, TT]]),
            )

        # sync ring: W1 weights + input rows in first-use order, then all
        # output writes. scalar ring: biases, fused W2, first dw2 chunks;
        # the rest of dw2 streams in from inside dec2(0) (time-gated).
        w1sb = [singles.tile([128, NP[e], 2, 4, 128], F8, name=f"w1sb{e}")
                for e in range(NE)]
        b1sb = singles.tile([128, NE * 4], F32)
        nc.scalar.dma_start(out=b1sb, in_=b1b[:, :])
        cbsb = singles.tile([128, NE * 4], F32)
        nc.scalar.dma_start(out=cbsb, in_=cb[:, :])
        # per-expert w2 slices in first-use order, interleaved with the first
        # dw2 chunks (dec2(0) starts early and needs chunk 0 by ~15us)
        w2sb = singles.tile([128, NE, 2, 2, 4, 128], F8)
        dw2sb = singles.tile([128, NCH, 4, 512], BF16)
        w2_order = []
        for segl in tile_segs:
            for (e, _, _) in segl:
                if e not in w2_order:
                    w2_order.append(e)
        w2_order += [e for e in range(NE) if e not in w2_order]
        nc.scalar.dma_start(out=w2sb[:, w2_order[0]],
                            in_=w2s[:, w2_order[0] * 2048: (w2_order[0] + 1) * 2048])
        for n in range(2):
            nc.scalar.dma_start(out=dw2sb[:, n, :, :],
                                in_=dw2s[:, n * 2048: (n + 1) * 2048])
        for e in w2_order[1:]:
            nc.scalar.dma_start(out=w2sb[:, e],
                                in_=w2s[:, e * 2048: (e + 1) * 2048])

        loaded_w, loaded_q = set(), set()

        def load_missing(pairs, q, max_block):
            run = []
            for p in sorted(pairs):
                if (p, q) in loaded_q:
                    continue
                loaded_q.add((p, q))
                if run and (p != run[-1] + 1 or len(run) >= max_block):
                    load_block(run[0], run[-1] + 1, q)
                    run = []
                run.append(p)
            if run:
                load_block(run[0], run[-1] + 1, q)

        e0_ = tile_segs[0][0][0]
        nc.sync.dma_start(out=w1sb[e0_][:, :1], in_=w1s[e0_][:, :1024])
        load_missing(range(PLO[e0_], PLO[e0_] + 2), 0, 2)
        nc.sync.dma_start(out=w1sb[e0_][:, 1:], in_=w1s[e0_][:, 1024:])
        load_missing(range(PLO[e0_], PLO[e0_] + NP[e0_]), 0, 2)
        loaded_w.add(e0_)
        for t in range(1, NT):
            for (e, _, _) in tile_segs[t]:
                if e not in loaded_w:
                    nc.sync.dma_start(out=w1sb[e], in_=w1s[e][:, :])
                    loaded_w.add(e)
                load_missing(range(PLO[e], PLO[e] + NP[e]), t, NPAIR)
        for e in range(NE):
            if e not in loaded_w:
                nc.sync.dma_start(out=w1sb[e], in_=w1s[e][:, :])

        sig_tiles = [None] * NT

        def emit_front(t):
            """W1 -> relu -> fused (W2 @ dec_W1) -> sigmoid for tile t."""
            selB = [psB.tile([128, TT], F32, name="selps") for _ in range(4)]
            for (e, lo, hi) in tile_segs[t]:
                w = hi - lo
                glo = t * TT + lo
                hpair = [htP.tile([128, 2, TT], F8, name="ht") for _ in range(2)]
                for m in range(4):
                    hps = psA.tile([128, TT], F32, name="hps")
                    for kp in range(NP[e]):
                        nc.tensor.matmul(
                            hps[:, :w],
                            w1sb[e][:, kp, :, m, :],
                            flat_all[:, PLO[e] + kp, :, glo: glo + w],
                            start=(kp == 0),
                            stop=(kp == NP[e] - 1),
                            perf_mode=DR,
                        )
                    # psum holds XS*WS*(x@W1); write HS*relu(x@W1 + b1) as fp8
                    nc.scalar.activation(
                        hpair[m // 2][:, m % 2, :w], hps[:, :w], AF.Relu,
                        bias=b1sb[:, e * 4 + m: e * 4 + m + 1],
                        scale=HS / (XS * WS),
                    )
                    if m % 2 == 1:
                        kp = m // 2
                        for md in range(4):
                            nc.tensor.matmul(
                                selB[md][:, lo:hi],
                                w2sb[:, e, kp, :, md, :],
                                hpair[kp][:, :, :w],
                                start=(kp == 0),
                                stop=(kp == 1),
                                perf_mode=DR,
                            )
            sigT = []
            for md in range(4):
                sg = sigP.tile([128, TT], BF16, name="sg")
                for (e, lo, hi) in tile_segs[t]:
                    nc.scalar.activation(
                        sg[:, lo:hi], selB[md][:, lo:hi], AF.Sigmoid,
                        bias=cbsb[:, e * 4 + md: e * 4 + md + 1],
                        scale=1.0 / (HS * MS),
                    )
                sigT.append(sg)
            sig_tiles[t] = sigT

        def emit_dec2(t):
            sigT = sig_tiles[t]
            for n in range(NCH):
                nw = min(512, NCLS - n * 512)
                if t == 0 and n + 2 < NCH:
                    # stream the rest of dw2 in from inside the first dec2
                    # pass: the scalar queue only reaches this once the
                    # preceding output writes fire, so these loads cannot
                    # steal HBM bandwidth from the startup-critical loads.
                    nc.scalar.dma_start(
                        out=dw2sb[:, n + 2, :, :],
                        in_=dw2s[:, (n + 2) * 2048: (n + 3) * 2048])
                ot = outP.tile([128, 4, 512], BF16, name="ot")
                eng = nc.sync if n % 2 == 0 else nc.scalar
                for s in range(TT // 128):
                    ps = psC.tile([128, 512], F32, name="d2ps")
                    for kh in range(4):
                        nc.tensor.matmul(
                            ps[:, :nw],
                            sigT[kh][:, s * 128: (s + 1) * 128],
                            dw2sb[:, n, kh, :nw],
                            start=(kh == 0),
                            stop=(kh == 3),
                        )
                    nc.vector.tensor_copy(out=ot[:, s, :nw], in_=ps[:, :nw])
                    if t == NT - 1:
                        # last tile: per-chunk writes so the final drain only
                        # trails the last cast by one small transfer
                        eng.dma_start(
                            out=out[t * TT + s * 128: t * TT + (s + 1) * 128,
                                    n * 512: n * 512 + nw],
                            in_=ot[:, s, :nw],
                        )
                if t < NT - 1:
                    # one merged write per (tile, column chunk)
                    eng.dma_start(
                        out=bass.AP(tensor=out,
                                    offset=(t * TT) * NCLS + n * 512,
                                    ap=[[NCLS, 128], [128 * NCLS, 4], [1, nw]]),
                        in_=ot[:, :, :nw],
                    )

        # dec2(t) directly after front(t): dec2 is a long dense PE stretch
        # whose inputs are already on-chip, giving the DMA rings a wide
        # window to stage the next tile's weights/activations.
        for t in range(NT):
            emit_front(t)
            emit_dec2(t)

    import bass_rust

    bass_rust.generate_event_semaphores(nc)
    return nc


_NC_CACHE = {}


def _get_nc(bounds):
    key = tuple(bounds)
    if key not in _NC_CACHE:
        _NC_CACHE[key] = _build(key)
    return _NC_CACHE[key]


def _swizzle_dr(w, s):
    """[K, M] f32 -> DoubleRow lhsT layout [128, (K/256)*2*(M/128)*128] fp8:
    element [p, kp, r, m, j] = s * w[(2*kp+r)*128+p, m*128+j]."""
    K, M = w.shape
    return np.ascontiguousarray(
        (w * s).reshape(K // 256, 2, 128, M // 128, 128)
        .transpose(2, 0, 1, 3, 4).reshape(128, (K // 128) * M).astype(f8)
    )


def _prep(inputs):
    f32 = np.float32
    x = np.asarray(inputs["fusion_hs"], f32)                 # [L, B, D]
    flat = np.ascontiguousarray(np.transpose(x, (1, 0, 2)).reshape(B, L * D))

    logits = flat.astype(np.float64) @ np.asarray(inputs["gate_W"], f32).astype(
        np.float64
    ) + np.asarray(inputs["gate_b"], f32).astype(np.float64)
    am = np.argmax(logits, axis=1)

    idx = [np.nonzero(am == e)[0] for e in range(NE)]
    # per-core per-expert slot counts: multiples of 128 summing to NDEV,
    # largest-remainder so tile boundaries mostly align with expert
    # boundaries. Tokens that don't fit their expert's slots run on the host.
    want = [len(idx[e]) / NCORES / 128 for e in range(NE)]
    ke = [int(w) for w in want]
    while sum(ke) < NDEV // 128:
        ke[max(range(NE), key=lambda e: want[e] - ke[e])] += 1
    ke = [k * 128 for k in ke]
    ke_dev = [min(ke[e], len(idx[e]) // NCORES) for e in range(NE)]
    bounds = (ke[0], ke[0] + ke[1], ke[0] + ke[1] + ke[2], NDEV)

    w1_3s = np.array(inputs["e3_W1"], f32, copy=True)
    w1_3s[: 3 * D] *= f32(np.asarray(inputs["e3_a"]).reshape(-1)[0])
    w1_3s[3 * D:] *= f32(np.asarray(inputs["e3_b"]).reshape(-1)[0])
    W1 = [np.asarray(inputs["e0_W1"], f32), np.asarray(inputs["e1_W1"], f32),
          np.asarray(inputs["e2_W1"], f32), w1_3s]
    W2 = [np.asarray(inputs[f"e{e}_W2"], f32) for e in range(NE)]
    b1 = [np.asarray(inputs[f"e{e}_b1"], f32) for e in range(NE)]
    b2 = [np.asarray(inputs[f"e{e}_b2"], f32) for e in range(NE)]
    dW1 = np.asarray(inputs["dec_W1"], f32)
    db1 = np.asarray(inputs["dec_b1"], f32)
    dW2 = np.asarray(inputs["dec_W2"], f32)
    db2 = np.asarray(inputs["dec_b2"], f32)

    # fuse the two back-to-back linear layers: sel@dW1 = h@(W2@dW1) + (b2@dW1)
    M2 = [W2[e] @ dW1 for e in range(NE)]
    cbv = [b2[e] @ dW1 + db1 for e in range(NE)]
    dW2p = np.zeros((512, NCH * 512), f32)
    dW2p[:, :NCLS] = dW2
    common = {
        "w1s0": _swizzle_dr(W1[0], WS), "w1s1": _swizzle_dr(W1[1], WS),
        "w1s2": _swizzle_dr(W1[2], WS), "w1s3": _swizzle_dr(W1[3], WS),
        "w2s": np.concatenate([_swizzle_dr(m, MS) for m in M2], axis=1),
        "dw2s": np.ascontiguousarray(
            dW2p.reshape(4, 128, NCH, 512).transpose(1, 2, 0, 3)
            .reshape(128, NCH * 2048).astype(bf16)
        ),
        # relu activation computes relu(psum/(XS*WS/HS)... ) with bias HS*b1
        "b1b": np.stack([HS * b1[e][m * 128: (m + 1) * 128]
                         for e in range(NE) for m in range(4)], axis=1),
        "cb": np.stack([cbv[e][m * 128: (m + 1) * 128]
                        for e in range(NE) for m in range(4)], axis=1),
    }

    perms, in_maps = [], []
    for c in range(NCORES):
        src = np.full(NDEV, -1, np.int64)
        base = 0
        for e in range(NE):
            k = ke_dev[e]
            src[base: base + k] = idx[e][c * k: (c + 1) * k]
            base += ke[e]
        perms.append(src)
        valid = src >= 0
        a = np.zeros((NDEV, L * D), f8)
        a[valid] = (flat[src[valid]] * XS).astype(f8)
        m = dict(common)
        m["fusionT"] = np.ascontiguousarray(a.T)
        in_maps.append(m)

    # overflow tokens: full forward on host in fp32
    lt = np.concatenate([idx[e][NCORES * ke_dev[e]:] for e in range(NE)])
    lt_out = np.zeros((len(lt), NCLS), f32)
    if len(lt):
        off = 0
        ins = [flat[:, : 3 * D], flat[:, 3 * D:], flat, flat]
        for e in range(NE):
            g = idx[e][NCORES * ke_dev[e]:]
            if len(g) == 0:
                continue
            h = np.maximum(ins[e][g] @ W1[e] + b1[e], 0)
            sel = h @ W2[e] + b2[e]
            sig = 1.0 / (1.0 + np.exp(-(sel @ dW1 + db1)))
            lt_out[off: off + len(g)] = sig @ dW2 + db2
            off += len(g)

    return bounds, in_maps, perms, lt, lt_out, db2


def kernel(**inputs):
    bounds, in_maps, perms, lt, lt_out, db2 = _prep(inputs)
    nc = _get_nc(bounds)
    res = run_bass_kernel_spmd(nc, in_maps, core_ids=list(range(NCORES)))
    out = np.empty((B, NCLS), np.float32)
    for c in range(NCORES):
        dev = np.asarray(res.results[c]["out"], np.float32)
        valid = perms[c] >= 0
        out[perms[c][valid]] = dev[valid] + db2
    if len(lt):
        out[lt] = lt_out
    return out


# revision 34
# speedup vs baseline: 1.0171x; 1.0154x over previous
import numpy as np
import ml_dtypes
from contextlib import ExitStack

import concourse.mybir as mybir
import concourse.bass as bass
import concourse.tile as tile
from concourse.bass_utils import run_bass_kernel_spmd

# Problem: nn_Predictor (moe_routing). L=6 streams, B=16384, D=512, NC=3992, 4 experts.
#
# Host computes the hard gate (argmax of fp64 logits) and expert-sorts tokens;
# each core gets 128-aligned per-expert slot counts (identical across cores ->
# one SPMD program); tokens that don't fit run on the host (<3%). Each token
# only runs through its own expert. Inputs arrive pre-transposed feature-major.
#
# Device pipeline per 512-token tile:
#   W1 (fp8 DoubleRow) -> relu -> fused W2@dec_W1 (fp8 DoubleRow) -> sigmoid
#   -> dec2 (bf16) -> out (bf16, +dec_b2 added on host)
# The two decoder linears are fused host-side (no nonlinearity between them).
# fp8 operands carry power-of-2 scales folded into the weights; the scales are
# removed for free via the activation instructions' `scale` argument.
L, B, D, NCLS, NE = 6, 16384, 512, 3992, 4
NCORES = 8
NDEV = 2048                 # device tokens per core (4 tiles of 512)
TT = 512
NT = NDEV // TT
NK = [12, 12, 24, 24]       # 128-wide K chunks of W1 per expert
NP = [k // 2 for k in NK]   # 256-wide DoubleRow K pairs
PLO = [0, 6, 0, 0]          # first fusionT chunk-pair each expert reads
NCH = (NCLS + 511) // 512   # 8 output column chunks (last = 408)

XS, WS, HS, MS = 16.0, 64.0, 32.0, 128.0   # fp8 scales: x, W1, h, W2@dW1

F32 = mybir.dt.float32
BF16 = mybir.dt.bfloat16
F8 = mybir.dt.float8e4
bf16 = ml_dtypes.bfloat16
f8 = ml_dtypes.float8_e4m3fn
AF = mybir.ActivationFunctionType
DR = mybir.MatmulPerfMode.DoubleRow


def _build(bounds):
    """bounds: cumulative token boundaries of the 4 expert segments in each
    core's 2048-token stream (identical for every core)."""
    nc = bass.Bass("TRN2")

    fusionT = nc.dram_tensor("fusionT", [L * D, NDEV], F8, kind="ExternalInput")
    w1s = [
        nc.dram_tensor(f"w1s{e}", [128, NP[e] * 1024], F8, kind="ExternalInput")
        for e in range(NE)
    ]
    w2s = nc.dram_tensor("w2s", [128, NE * 2048], F8, kind="ExternalInput")
    dw2s = nc.dram_tensor("dw2s", [128, NCH * 2048], BF16, kind="ExternalInput")
    b1b = nc.dram_tensor("b1b", [128, NE * 4], F32, kind="ExternalInput")
    cb = nc.dram_tensor("cb", [128, NE * 4], F32, kind="ExternalInput")
    out = nc.dram_tensor("out", [NDEV, NCLS], BF16, kind="ExternalOutput")

    segs = []
    prev = 0
    for e, b in enumerate(bounds):
        if b > prev:
            segs.append((e, prev, b))
            prev = b
    tile_segs = []
    for t in range(NT):
        t0, t1 = t * TT, (t + 1) * TT
        tile_segs.append(
            [(e, max(lo, t0) - t0, min(hi, t1) - t0) for (e, lo, hi) in segs
             if lo < t1 and hi > t0]
        )

    with tile.TileContext(nc) as tc, ExitStack() as ctx:
        singles = ctx.enter_context(tc.tile_pool(name="singles", bufs=1))
        htP = ctx.enter_context(tc.tile_pool(name="htP", bufs=6))
        sigP = ctx.enter_context(tc.tile_pool(name="sigP", bufs=6))
        outP = ctx.enter_context(tc.tile_pool(name="outP", bufs=6))

        psA = ctx.enter_context(tc.tile_pool(name="psA", bufs=2, space="PSUM"))
        psB = ctx.enter_context(tc.tile_pool(name="psB", bufs=4, space="PSUM"))
        psC = ctx.enter_context(tc.tile_pool(name="psC", bufs=2, space="PSUM"))

        # every 256-feature chunk pair is loaded once, whole-row (all 2048
        # tokens), and stays resident; all tiles slice their token columns.
        # Loads are merged into block DMAs (each dma_start costs ~650ns of
        # engine issue time) and column-halved so tiles 0/1 start early.
        NPAIR = L * D // 256
        flat_all = singles.tile([128, NPAIR, 2, NDEV], F8)

        def load_block(p0, p1, q):
            # pairs [p0,p1) x token columns [q*TT, (q+1)*TT) only
            nc.sync.dma_start(
                out=flat_all[:, p0:p1, :, q * TT: (q + 1) * TT],
                in_=bass.AP(
                    tensor=fusionT, offset=p0 * 256 * NDEV + q * TT,
                    ap=[[NDEV, 128], [256 * NDEV, p1 - p0], [128 * NDEV, 2],
                        [1, TT]]),
            )

        # sync ring: W1 weights + input rows in first-use order, then all
        # output writes. scalar ring: biases, fused W2, first dw2 chunks;
        # the rest of dw2 streams in from inside dec2(0) (time-gated).
        w1sb = [singles.tile([128, NP[e], 2, 4, 128], F8, name=f"w1sb{e}")
                for e in range(NE)]
        b1sb = singles.tile([128, NE * 4], F32)
        nc.scalar.dma_start(out=b1sb, in_=b1b[:, :])
        cbsb = singles.tile([128, NE * 4], F32)
        nc.scalar.dma_start(out=cbsb, in_=cb[:, :])
        # per-expert w2 slices in first-use order, interleaved with the first
        # dw2 chunks (dec2(0) starts early and needs chunk 0 by ~15us)
        w2sb = singles.tile([128, NE, 2, 2, 4, 128], F8)
        dw2sb = singles.tile([128, NCH, 4, 512], BF16)
        w2_order = []
        for segl in tile_segs:
            for (e, _, _) in segl:
                if e not in w2_order:
                    w2_order.append(e)
        w2_order += [e for e in range(NE) if e not in w2_order]
        nc.scalar.dma_start(out=w2sb[:, w2_order[0]],
                            in_=w2s[:, w2_order[0] * 2048: (w2_order[0] + 1) * 2048])
        for n in range(2):
            nc.scalar.dma_start(out=dw2sb[:, n, :, :],
                                in_=dw2s[:, n * 2048: (n + 1) * 2048])
        for e in w2_order[1:]:
            nc.scalar.dma_start(out=w2sb[:, e],
                                in_=w2s[:, e * 2048: (e + 1) * 2048])

        loaded_w, loaded_q = set(), set()

        def load_missing(pairs, q, max_block):
            run = []
            for p in sorted(pairs):
                if (p, q) in loaded_q:
                    continue
                loaded_q.add((p, q))
                if run and (p != run[-1] + 1 or len(run) >= max_block):
                    load_block(run[0], run[-1] + 1, q)
                    run = []
                run.append(p)
            if run:
                load_block(run[0], run[-1] + 1, q)

        e0_ = tile_segs[0][0][0]
        nc.sync.dma_start(out=w1sb[e0_][:, :1], in_=w1s[e0_][:, :1024])
        load_missing(range(PLO[e0_], PLO[e0_] + 2), 0, 2)
        nc.sync.dma_start(out=w1sb[e0_][:, 1:], in_=w1s[e0_][:, 1024:])
        load_missing(range(PLO[e0_], PLO[e0_] + NP[e0_]), 0, 2)
        loaded_w.add(e0_)
        for t in range(1, NT):
            for (e, _, _) in tile_segs[t]:
                if e not in loaded_w:
                    nc.sync.dma_start(out=w1sb[e], in_=w1s[e][:, :])
                    loaded_w.add(e)
                load_missing(range(PLO[e], PLO[e] + NP[e]), t, NPAIR)
        for e in range(NE):
            if e not in loaded_w:
                nc.sync.dma_start(out=w1sb[e], in_=w1s[e][:, :])

        sig_tiles = [None] * NT

        def emit_front(t):
            """W1 -> relu -> fused (W2 @ dec_W1) -> sigmoid for tile t."""
            selB = [psB.tile([128, TT], F32, name="selps") for _ in range(4)]
            for (e, lo, hi) in tile_segs[t]:
                w = hi - lo
                glo = t * TT + lo
                hpair = [htP.tile([128, 2, TT], F8, name="ht") for _ in range(2)]
                for m in range(4):
                    hps = psA.tile([128, TT], F32, name="hps")
                    for kp in range(NP[e]):
                        nc.tensor.matmul(
                            hps[:, :w],
                            w1sb[e][:, kp, :, m, :],
                            flat_all[:, PLO[e] + kp, :, glo: glo + w],
                            start=(kp == 0),
                            stop=(kp == NP[e] - 1),
                            perf_mode=DR,
                        )
                    # psum holds XS*WS*(x@W1); write HS*relu(x@W1 + b1) as fp8
                    nc.scalar.activation(
                        hpair[m // 2][:, m % 2, :w], hps[:, :w], AF.Relu,
                        bias=b1sb[:, e * 4 + m: e * 4 + m + 1],
                        scale=HS / (XS * WS),
                    )
                    if m % 2 == 1:
                        kp = m // 2
                        for md in range(4):
                            nc.tensor.matmul(
                                selB[md][:, lo:hi],
                                w2sb[:, e, kp, :, md, :],
                                hpair[kp][:, :, :w],
                                start=(kp == 0),
                                stop=(kp == 1),
                                perf_mode=DR,
                            )
            sigT = []
            for md in range(4):
                sg = sigP.tile([128, TT], BF16, name="sg")
                for (e, lo, hi) in tile_segs[t]:
                    nc.scalar.activation(
                        sg[:, lo:hi], selB[md][:, lo:hi], AF.Sigmoid,
                        bias=cbsb[:, e * 4 + md: e * 4 + md + 1],
                        scale=1.0 / (HS * MS),
                    )
                sigT.append(sg)
            sig_tiles[t] = sigT

        def emit_dec2(t):
            sigT = sig_tiles[t]
            for n in range(NCH):
                nw = min(512, NCLS - n * 512)
                if t == 0 and n + 2 < NCH:
                    # stream the rest of dw2 in from inside the first dec2
                    # pass: the scalar queue only reaches this once the
                    # preceding output writes fire, so these loads cannot
                    # steal HBM bandwidth from the startup-critical loads.
                    nc.scalar.dma_start(
                        out=dw2sb[:, n + 2, :, :],
                        in_=dw2s[:, (n + 2) * 2048: (n + 3) * 2048])
                ot = outP.tile([128, 4, 512], BF16, name="ot")
                eng = nc.sync if n % 2 == 0 else nc.scalar
                for s in range(TT // 128):
                    ps = psC.tile([128, 512], F32, name="d2ps")
                    for kh in range(4):
                        nc.tensor.matmul(
                            ps[:, :nw],
                            sigT[kh][:, s * 128: (s + 1) * 128],
                            dw2sb[:, n, kh, :nw],
                            start=(kh == 0),
                            stop=(kh == 3),
                        )
                    nc.vector.tensor_copy(out=ot[:, s, :nw], in_=ps[:, :nw])
                    if t == NT - 1:
                        # last tile: per-chunk writes so the final drain only
                        # trails the last cast by one small transfer
                        eng.dma_start(
                            out=out[t * TT + s * 128: t * TT + (s + 1) * 128,
                                    n * 512: n * 512 + nw],
                            in_=ot[:, s, :nw],
                        )
                if t < NT - 1:
                    # one merged write per (tile, column chunk)
                    eng.dma_start(
                        out=bass.AP(tensor=out,
                                    offset=(t * TT) * NCLS + n * 512,
                                    ap=[[NCLS, 128], [128 * NCLS, 4], [1, nw]]),
                        in_=ot[:, :, :nw],
                    )

        # software pipeline: dec2 of tile t-1 (a long PE stretch whose inputs
        # are already on-chip) runs while tile t's sigmoid copies land.
        for t in range(NT):
            emit_front(t)
            if t > 0:
                emit_dec2(t - 1)
        emit_dec2(NT - 1)

    import bass_rust

    bass_rust.generate_event_semaphores(nc)
    return nc


_NC_CACHE = {}


def _get_nc(bounds):
    key = tuple(bounds)
    if key not in _NC_CACHE:
        _NC_CACHE[key] = _build(key)
    return _NC_CACHE[key]


def _swizzle_dr(w, s):
    """[K, M] f32 -> DoubleRow lhsT layout [128, (K/256)*2*(M/128)*128] fp8:
    element [p, kp, r, m, j] = s * w[(2*kp+r)*128+p, m*128+j]."""
    K, M = w.shape
    return np.ascontiguousarray(
        (w * s).reshape(K // 256, 2, 128, M // 128, 128)
        .transpose(2, 0, 1, 3, 4).reshape(128, (K // 128) * M).astype(f8)
    )


def _prep(inputs):
    f32 = np.float32
    x = np.asarray(inputs["fusion_hs"], f32)                 # [L, B, D]
    flat = np.ascontiguousarray(np.transpose(x, (1, 0, 2)).reshape(B, L * D))

    logits = flat.astype(np.float64) @ np.asarray(inputs["gate_W"], f32).astype(
        np.float64
    ) + np.asarray(inputs["gate_b"], f32).astype(np.float64)
    am = np.argmax(logits, axis=1)

    idx = [np.nonzero(am == e)[0] for e in range(NE)]
    # per-core per-expert slot counts: multiples of 128 summing to NDEV,
    # largest-remainder so tile boundaries mostly align with expert
    # boundaries. Tokens that don't fit their expert's slots run on the host.
    want = [len(idx[e]) / NCORES / 128 for e in range(NE)]
    ke = [int(w) for w in want]
    while sum(ke) < NDEV // 128:
        ke[max(range(NE), key=lambda e: want[e] - ke[e])] += 1
    ke = [k * 128 for k in ke]
    ke_dev = [min(ke[e], len(idx[e]) // NCORES) for e in range(NE)]
    bounds = (ke[0], ke[0] + ke[1], ke[0] + ke[1] + ke[2], NDEV)

    w1_3s = np.array(inputs["e3_W1"], f32, copy=True)
    w1_3s[: 3 * D] *= f32(np.asarray(inputs["e3_a"]).reshape(-1)[0])
    w1_3s[3 * D:] *= f32(np.asarray(inputs["e3_b"]).reshape(-1)[0])
    W1 = [np.asarray(inputs["e0_W1"], f32), np.asarray(inputs["e1_W1"], f32),
          np.asarray(inputs["e2_W1"], f32), w1_3s]
    W2 = [np.asarray(inputs[f"e{e}_W2"], f32) for e in range(NE)]
    b1 = [np.asarray(inputs[f"e{e}_b1"], f32) for e in range(NE)]
    b2 = [np.asarray(inputs[f"e{e}_b2"], f32) for e in range(NE)]
    dW1 = np.asarray(inputs["dec_W1"], f32)
    db1 = np.asarray(inputs["dec_b1"], f32)
    dW2 = np.asarray(inputs["dec_W2"], f32)
    db2 = np.asarray(inputs["dec_b2"], f32)

    # fuse the two back-to-back linear layers: sel@dW1 = h@(W2@dW1) + (b2@dW1)
    M2 = [W2[e] @ dW1 for e in range(NE)]
    cbv = [b2[e] @ dW1 + db1 for e in range(NE)]
    dW2p = np.zeros((512, NCH * 512), f32)
    dW2p[:, :NCLS] = dW2
    common = {
        "w1s0": _swizzle_dr(W1[0], WS), "w1s1": _swizzle_dr(W1[1], WS),
        "w1s2": _swizzle_dr(W1[2], WS), "w1s3": _swizzle_dr(W1[3], WS),
        "w2s": np.concatenate([_swizzle_dr(m, MS) for m in M2], axis=1),
        "dw2s": np.ascontiguousarray(
            dW2p.reshape(4, 128, NCH, 512).transpose(1, 2, 0, 3)
            .reshape(128, NCH * 2048).astype(bf16)
        ),
        # relu activation computes relu(psum/(XS*WS/HS)... ) with bias HS*b1
        "b1b": np.stack([HS * b1[e][m * 128: (m + 1) * 128]
                         for e in range(NE) for m in range(4)], axis=1),
        "cb": np.stack([cbv[e][m * 128: (m + 1) * 128]
                        for e in range(NE) for m in range(4)], axis=1),
    }

    perms, in_maps = [], []
    for c in range(NCORES):
        src = np.full(NDEV, -1, np.int64)
        base = 0
        for e in range(NE):
            k = ke_dev[e]
            src[base: base + k] = idx[e][c * k: (c + 1) * k]
            base += ke[e]
        perms.append(src)
        valid = src >= 0
        a = np.zeros((NDEV, L * D), f8)
        a[valid] = (flat[src[valid]] * XS).astype(f8)
        m = dict(common)
        m["fusionT"] = np.ascontiguousarray(a.T)
        in_maps.append(m)

    # overflow tokens: full forward on host in fp32
    lt = np.concatenate([idx[e][NCORES * ke_dev[e]:] for e in range(NE)])
    lt_out = np.zeros((len(lt), NCLS), f32)
    if len(lt):
        off = 0
        ins = [flat[:, : 3 * D], flat[:, 3 * D:], flat, flat]
        for e in range(NE):
            g = idx[e][NCORES * ke_dev[e]:]
            if len(g) == 0:
                continue
            h = np.maximum(ins[e][g] @ W1[e] + b1[e], 0)
            sel = h @ W2[e] + b2[e]
            sig = 1.0 / (1.0 + np.exp(-(sel @ dW1 + db1)))
            lt_out[off: off + len(g)] = sig @ dW2 + db2
            off += len(g)

    return bounds, in_maps, perms, lt, lt_out, db2


def kernel(**inputs):
    bounds, in_maps, perms, lt, lt_out, db2 = _prep(inputs)
    nc = _get_nc(bounds)
    res = run_bass_kernel_spmd(nc, in_maps, core_ids=list(range(NCORES)))
    out = np.empty((B, NCLS), np.float32)
    for c in range(NCORES):
        dev = np.asarray(res.results[c]["out"], np.float32)
        valid = perms[c] >= 0
        out[perms[c][valid]] = dev[valid] + db2
    if len(lt):
        out[lt] = lt_out
    return out
